# revision 16
# baseline (speedup 1.0000x reference)
"""ONI-Norm TRN2 kernel v6: float32r z path, bf16 gram, no cast pass.

vs v4 baseline (190us):
- The whole z path is declared float32r (identical bits to fp32, no cast
  pass needed): PE transposes run at 1.5 cyc/row (vs 2.0 fp32) and the
  projection matmul runs at 1 cyc/row at moving width 512 (vs 4 for fp32)
  with near-fp32 precision.
- Gram matmuls in bf16 (1 cyc/row): the transposed slices are cast to
  bf16 during the mandatory PSUM->SBUF copy, which is load-balanced
  across DVE / ACT / GpSimd (measured rates ~1.35 / 0.72 / 1.8 us per
  512-col unit).
- Each transposed slice carries a ones column, so the Gram matmul also
  accumulates per-row sums into PSUM col 128 -- no row-sum reduces.
  The ones columns are written once per zt pool buffer; PE program order
  guarantees they are set before any gram matmul reads them.
- Newton-Schulz restructured depth-2 per iteration: X=B@B and Y=B@Sh are
  independent (all iterates are polynomials in Sn, so they commute), then
  B' = 1.5B - X@Y via one scalar_tensor_tensor. NS stays fp32.
- Projection of group 0 is emitted interleaved with the tail of group 1's
  Gram so stores start ~30us in.
"""

import math
from contextlib import ExitStack

import numpy as np

import concourse.bacc as bacc
import concourse.mybir as mybir
from concourse.bass import ds, ts, MemorySpace
from concourse.bass_isa import ReduceOp
from concourse.bass_utils import run_bass_kernel_spmd
from concourse.masks import make_identity
from concourse.tile import TileContext

P = 128
K = 18432
G_TOTAL = 16
N_CORES = 8
G_PER_CORE = G_TOTAL // N_CORES
ROWS_PER_CORE = G_PER_CORE * P
T_NS = 5
EPS = 1e-5
CHUNK = 2048
N_CHUNKS = K // CHUNK
SUB = 512
SUB_PER_CHUNK = CHUNK // SUB
N_SLICES = N_CHUNKS * SUB_PER_CHUNK   # 36 per group
F32 = mybir.dt.float32
F32R = mybir.dt.float32r
BF16 = mybir.dt.bfloat16
AUG = P + 1  # 129: gram moving width with the ones column
ZT_BUFS = 4


def build_nc():
    nc = bacc.Bacc("TRN2", target_bir_lowering=False)
    x = nc.dram_tensor("x", [ROWS_PER_CORE, K], F32R, kind="ExternalInput")
    y = nc.dram_tensor("y", [ROWS_PER_CORE, K], F32, kind="ExternalOutput")

    with TileContext(nc) as tc, ExitStack() as ctx:
        consts = ctx.enter_context(tc.tile_pool(name="consts", bufs=1))
        identity = consts.tile([P, P], F32)
        make_identity(nc, identity)
        identity_r = consts.tile([P, P], F32R)
        nc.vector.tensor_copy(identity_r, identity)
        eps_eye = consts.tile([P, P], F32)
        nc.vector.tensor_scalar_mul(eps_eye, identity, EPS)
        ones = consts.tile([P, P], F32)
        nc.any.memset(ones, 1.0)

        zpool = ctx.enter_context(tc.tile_pool(name="z", bufs=G_PER_CORE * N_CHUNKS))
        ztp = ctx.enter_context(tc.tile_pool(name="zt", bufs=ZT_BUFS))
        outp = ctx.enter_context(tc.tile_pool(name="out", bufs=3))
        nsp = ctx.enter_context(tc.tile_pool(name="ns", bufs=2))
        vecp = ctx.enter_context(tc.tile_pool(name="vec", bufs=2))
        ps_S = ctx.enter_context(tc.tile_pool(name="psS", bufs=1, space=MemorySpace.PSUM))
        ps_T = ctx.enter_context(tc.tile_pool(name="psT", bufs=2, space=MemorySpace.PSUM))
        ps_proj = ctx.enter_context(tc.tile_pool(name="psP", bufs=2, space=MemorySpace.PSUM))
        ps_ns = ctx.enter_context(tc.tile_pool(name="psN", bufs=2, space=MemorySpace.PSUM))
        ps_vec = ctx.enter_context(tc.tile_pool(name="psV", bufs=1, space=MemorySpace.PSUM))

        st = [dict() for _ in range(G_PER_CORE)]

        def emit_loads(g):
            s = st[g]
            s["zs"] = []
            for c in range(N_CHUNKS):
                z = zpool.tile([P, CHUNK], F32R, tag="z", name=f"z{g}_{c}")
                if g == 0 and c == 0:
                    # smaller first transfers so the pipeline starts sooner
                    for t4 in range(SUB_PER_CHUNK):
                        nc.sync.dma_start(
                            z[:, ts(t4, SUB)],
                            x[ds(g * P, P), ds(c * CHUNK + t4 * SUB, SUB)],
                        )
                else:
                    nc.sync.dma_start(z, x[ds(g * P, P), ts(c, CHUNK)])
                s["zs"].append(z)

        def emit_gram_T(g, si):
            """Transpose slice si (4x 128-col blocks) and cast-copy to bf16."""
            s = st[g]
            c, t = divmod(si, SUB_PER_CHUNK)
            tp = ps_T.tile([P, SUB_PER_CHUNK, P], F32R, tag="T", name=f"tp{g}_{si}")
            for b in range(SUB // P):
                nc.tensor.transpose(
                    tp[:, ds(b, 1), :],
                    s["zs"][c][:, ds(t * SUB + b * P, P)],
                    identity_r,
                )
            zt = ztp.tile([P, SUB // P, AUG], BF16, tag="zt", name=f"zt{g}_{si}")
            if g == 0 and si < ZT_BUFS:
                # ones columns survive buffer rotation: later tiles only
                # overwrite the data columns, and PE program order puts
                # every later gram matmul after the first-round ones
                nc.vector.memset(zt[:, :, ds(P, 1)], 1.0)
            # all on DVE: ACT is the hotter engine (output stage + store issue)
            nc.vector.tensor_copy(zt[:, :, ds(0, P)], tp.bitcast(F32))
            s.setdefault("zt_pend", {})[si] = zt

        def emit_gram_M(g, si):
            s = st[g]
            if si == 0:
                s["S_ps"] = ps_S.tile([P, AUG], F32, tag="S", name=f"Sps{g}")
            zt = s["zt_pend"].pop(si)
            for b in range(SUB // P):
                nc.tensor.matmul(
                    s["S_ps"],
                    zt[:, ds(b, 1), ds(0, P)],
                    zt[:, ds(b, 1), :],
                    start=(si == 0 and b == 0),
                    stop=(si == N_SLICES - 1 and b == SUB // P - 1),
                )

        def emit_gram_slice(g, si):
            # transposes of slice si, then matmuls of slice si-2 (2-slice lag
            # so the cast-copy is done before the PE needs the bf16 tile)
            emit_gram_T(g, si)
            if si >= 2:
                emit_gram_M(g, si - 2)
            if si == N_SLICES - 1:
                emit_gram_M(g, si - 1)
                emit_gram_M(g, si)

        def emit_mean_chain(g):
            s = st[g]
            # rowsum came for free from the ones column of the gram moving
            rsum = vecp.tile([P, 1], F32, name=f"rs{g}")
            nc.vector.tensor_copy(rsum, s["S_ps"][:, ds(P, 1)])
            mean = vecp.tile([P, 1], F32, name=f"mean{g}")
            nc.vector.tensor_scalar_mul(mean, rsum, 1.0 / K)
            s["mean"] = mean
            m12 = vecp.tile([P, 1], F32, name=f"m12{g}")
            nc.vector.tensor_scalar_mul(m12, rsum, math.sqrt(K / P) / K)
            Mm = vecp.tile([P, P], F32, name=f"Mm{g}")
            nc.vector.tensor_scalar_mul(Mm, ones, m12)
            M_ps = ps_vec.tile([P, P], F32, tag="v", name=f"Mps{g}")
            nc.tensor.matmul(M_ps, Mm, identity, start=True, stop=True)
            M128a = vecp.tile([P, P], F32, name=f"Ma{g}")
            nc.vector.tensor_copy(M128a, M_ps)
            M128b = vecp.tile([P, P], F32, name=f"Mb{g}")
            nc.vector.tensor_scalar_mul(M128b, M128a, -1.0)
            # accumulate the rank-1 mean correction onto the closed gram sum
            nc.tensor.matmul(
                s["S_ps"][:, ds(0, P)], M128a, M128b, start=False, stop=True
            )

            S = nsp.tile([P, P], F32, tag="S", name=f"S{g}")
            nc.vector.tensor_copy(S, s["S_ps"][:, ds(0, P)])
            nc.vector.tensor_add(S, S, eps_eye)
            S2 = nsp.tile([P, P], F32, tag="S2", name=f"S2_{g}")
            frob2 = vecp.tile([P, 1], F32, name=f"fr{g}")
            nc.scalar.activation(
                S2, S, mybir.ActivationFunctionType.Square, accum_out=frob2
            )
            nc.gpsimd.partition_all_reduce(frob2, frob2, P, ReduceOp.add)
            nu = vecp.tile([P, 1], F32, name=f"nu{g}")
            nc.scalar.sqrt(nu, frob2)
            inv_nu = vecp.tile([P, 1], F32, name=f"inu{g}")
            nc.vector.reciprocal(inv_nu, nu)
            oscale = vecp.tile([P, 1], F32, name=f"osc{g}")
            nc.scalar.sqrt(oscale, inv_nu)
            s["oscale"] = oscale
            Sn = nsp.tile([P, P], F32, tag="Sn", name=f"Sn{g}")
            nc.vector.tensor_scalar_mul(Sn, S, inv_nu)
            S_half = nsp.tile([P, P], F32, tag="Sh", name=f"Sh{g}")
            nc.vector.tensor_scalar_mul(S_half, Sn, 0.5)
            s["S_half"] = S_half
            B = nsp.tile([P, P], F32, tag=f"B{g}", name=f"B0_{g}")
            # B0 after one reference iteration: 1.5*I - 0.5*Sn
            nc.vector.scalar_tensor_tensor(
                B, identity, 1.5, S_half,
                op0=mybir.AluOpType.mult, op1=mybir.AluOpType.subtract,
            )
            s["B"] = B

        def emit_ns_step(g, it, sub):
            # depth-2 Newton-Schulz: X=B@B and Y=B@Sh are independent,
            # then B' = 1.5*B - X@Y  (all iterates commute with Sn)
            s = st[g]
            if sub == 0:
                x_ps = ps_ns.tile([P, P], F32, tag="ns", name=f"x{g}_{it}")
                nc.tensor.matmul(x_ps, s["B"], s["B"], start=True, stop=True)
                X = nsp.tile([P, P], F32, tag=f"X{g}", name=f"X{g}_{it}")
                nc.vector.tensor_copy(X, x_ps)
                s["X"] = X
            elif sub == 1:
                y_ps = ps_ns.tile([P, P], F32, tag="ns", name=f"y{g}_{it}")
                nc.tensor.matmul(y_ps, s["B"], s["S_half"], start=True, stop=True)
                Y = nsp.tile([P, P], F32, tag=f"Y{g}", name=f"Y{g}_{it}")
                nc.vector.tensor_copy(Y, y_ps)
                s["Y"] = Y
            else:
                z_ps = ps_ns.tile([P, P], F32, tag="ns", name=f"z{g}_{it}")
                nc.tensor.matmul(z_ps, s["X"], s["Y"], start=True, stop=True)
                Bn = nsp.tile([P, P], F32, tag=f"Bn{g}", name=f"Bn{g}_{it}")
                nc.vector.scalar_tensor_tensor(
                    Bn, s["B"], 1.5, z_ps,
                    op0=mybir.AluOpType.mult, op1=mybir.AluOpType.subtract,
                )
                s["B"] = Bn

        def emit_cbias(g):
            s = st[g]
            c_ps = ps_vec.tile([P, 1], F32, tag="v", name=f"cps{g}")
            # B is symmetric (polynomial in Sn) so lhsT=B gives B@mean
            nc.tensor.matmul(c_ps, s["B"], s["mean"], start=True, stop=True)
            negos = vecp.tile([P, 1], F32, name=f"ng{g}")
            nc.vector.tensor_scalar_mul(negos, s["oscale"], -1.0)
            bias = vecp.tile([P, 1], F32, name=f"bi{g}")
            nc.vector.tensor_mul(bias, negos, c_ps)
            s["bias"] = bias
            # fp32r "rounding" copy so the verifier accepts B as a fp32r
            # matmul operand
            B_r = nsp.tile([P, P], F32R, tag=f"Br{g}", name=f"Br{g}")
            nc.vector.tensor_copy(B_r, s["B"])
            s["B_r"] = B_r

        def emit_proj_slice(g, si):
            s = st[g]
            c, t = divmod(si, SUB_PER_CHUNK)
            if t == 0:
                s["out_t"] = outp.tile([P, CHUNK], F32, tag="out", name=f"o{g}_{c}")
            pr = ps_proj.tile([P, SUB], F32, tag="proj", name=f"pr{g}_{si}")
            nc.tensor.matmul(
                pr,
                s["B_r"],
                s["zs"][c][:, ts(t, SUB)],
                start=True, stop=True,
            )
            nc.scalar.activation(
                s["out_t"][:, ts(t, SUB)], pr,
                mybir.ActivationFunctionType.Identity,
                bias=s["bias"], scale=s["oscale"],
            )
            # stores ride the Activation engine's DGE rings so they are not
            # queued behind the load descriptors on the SP rings
            if g == G_PER_CORE - 1 and c == N_CHUNKS - 1:
                # smaller final stores so the tail drains pipelined
                nc.scalar.dma_start(
                    y[ds(g * P, P), ds(c * CHUNK + t * SUB, SUB)],
                    s["out_t"][:, ts(t, SUB)],
                )
            elif t == SUB_PER_CHUNK - 1:
                nc.scalar.dma_start(y[ds(g * P, P), ts(c, CHUNK)], s["out_t"])

        # ---------------- emission schedule ----------------
        emit_loads(0)
        emit_loads(1)
        for si in range(N_SLICES):
            emit_gram_slice(0, si)
        emit_mean_chain(0)

        # NS(g0) interleaved with the first 12 gram slices of g1
        g1_si = 0
        for it in range(T_NS - 1):
            for sub in range(3):
                emit_ns_step(0, it, sub)
                if g1_si < N_SLICES:
                    emit_gram_slice(1, g1_si)
                    g1_si += 1
        emit_cbias(0)

        # ALL of proj(g0) interleaved with the remaining gram slices of g1:
        # finishing proj(g0) inside g1's load window builds a store backlog
        # that keeps the DMA busy through the NS(g1) serial section (else
        # HAM halves the PE clock during a sparse tail)
        p0_si = 0
        while g1_si < N_SLICES:
            emit_gram_slice(1, g1_si)
            g1_si += 1
            emit_proj_slice(0, p0_si)
            p0_si += 1
        while p0_si < N_SLICES:
            emit_proj_slice(0, p0_si)
            p0_si += 1
        emit_mean_chain(1)
        for it in range(T_NS - 1):
            for sub in range(3):
                emit_ns_step(1, it, sub)
        emit_cbias(1)
        for si in range(N_SLICES):
            emit_proj_slice(1, si)

    nc.finalize()
    return nc


_NC_CACHE = None


def _get_nc():
    global _NC_CACHE
    if _NC_CACHE is None:
        _NC_CACHE = build_nc()
    return _NC_CACHE


def kernel(weight, _trace=False):
    w = np.ascontiguousarray(np.asarray(weight, dtype=np.float32))
    assert w.shape == (G_TOTAL * P, K), w.shape
    nc = _get_nc()
    in_maps = [
        {"x": np.ascontiguousarray(w[core * ROWS_PER_CORE:(core + 1) * ROWS_PER_CORE])}
        for core in range(N_CORES)
    ]
    res = run_bass_kernel_spmd(
        nc, in_maps, core_ids=list(range(N_CORES)), trace=_trace
    )
    out = np.concatenate([r["y"] for r in res.results], axis=0)
    if _trace:
        return out, res
    return out
